# revision 7
# baseline (speedup 1.0000x reference)
"""Trainium2 Bass kernel for a decoder layer (attention + EMA-smeared K + sinusoidal
pos-emb + causal softmax + output proj + residual + LayerNorm).

Sharding: tensor-parallel over heads for QKV+attention (2 heads/core x 8 cores,
both batches on every core), then an 8-rank AllToAll redistributes the attention
output so each core owns one (batch, seq-slice) for the output projection,
residual add and LayerNorm (Megatron sequence-parallel tail).

All matmuls run as float32r (full-rate PE path, ~1e-4 relative error).
"""

import numpy as np

import concourse.bass as bass
import concourse.mybir as mybir
import concourse.tile as tile
from concourse import bacc
from concourse.bass_utils import run_bass_kernel_spmd
from concourse.masks import make_identity

B, S, D, H = 2, 2048, 1024, 16
DK = D // H            # 64
NCORES = 8
HPC = H // NCORES      # heads per core = 2
DLOC = HPC * DK        # local attn dims = 128
SB = 512               # seq block (matmul N)
NSB = S // SB          # 4
KTILE = 128
NKT = S // KTILE       # 16
LN_EPS = 1e-5
f32 = mybir.dt.float32
f32r = mybir.dt.float32r
AX = mybir.AxisListType.X
AF = mybir.ActivationFunctionType


def _emit(nc, tc, P, ctx):
    """Emit one full forward pass."""
    import contextlib

    # float32r is a full-width 32-bit dtype (PE fast path), not a true
    # low-precision accumulate — silence the guard.
    ctx.enter_context(nc.allow_low_precision(reason="float32r PE operands"))

    # ---------------- constants ----------------
    cpool = ctx.enter_context(tc.tile_pool(name="const", bufs=1))
    peT = cpool.tile([128, S], f32)
    nc.sync.dma_start(peT[:], P["peT"][:])
    mask_sb = []
    for p in range(4):
        m = cpool.tile([KTILE, SB], f32, tag=f"mask{p}", name=f"mask{p}")
        nc.sync.dma_start(m[:], P["mask"][p])
        mask_sb.append(m)
    ident = cpool.tile([128, 128], f32)
    make_identity(nc, ident[:])
    ones_f = cpool.tile([128, HPC], f32)
    nc.vector.memset(ones_f[:], 1.0)
    ones64 = cpool.tile([1, DK], f32r)
    nc.vector.tensor_copy(ones64[:], ones_f[0:1, 0:1].to_broadcast((1, DK)))
    eps_t = cpool.tile([128, 1], f32)
    nc.vector.memset(eps_t[:], LN_EPS)
    lnsc = cpool.tile([128, D], f32)
    nc.sync.dma_start(lnsc[:], P["lnsc"][:])
    lnb = cpool.tile([128, D], f32)
    nc.sync.dma_start(lnb[:], P["lnb"][:])
    alphasig = cpool.tile([128, S - 1], f32)

    # pools that live through phases A+B
    qt_pool = ctx.enter_context(tc.tile_pool(name="qt", bufs=2))
    k2_pool = ctx.enter_context(tc.tile_pool(name="k2", bufs=2))
    vaug_pool = ctx.enter_context(tc.tile_pool(name="vaug", bufs=2 * NKT))
    QTf = [qt_pool.tile([128, S], f32r, tag="qtf", name="qtf") for _ in range(B)]
    K2 = [k2_pool.tile([128, S], f32r, tag="k2", name="k2") for _ in range(B)]
    vaug = [[vaug_pool.tile([128, HPC * (DK + 1)], f32r, tag="vaug", name="vaug")
             for _ in range(NKT)] for _ in range(B)]

    # ---------------- phase A: projections ----------------
    with contextlib.ExitStack() as actx:
        apool = actx.enter_context(tc.tile_pool(name="phA", bufs=2))
        xt_pool = actx.enter_context(tc.tile_pool(name="xt", bufs=8))
        w_pool = actx.enter_context(tc.tile_pool(name="wqkv", bufs=1))
        psA = actx.enter_context(tc.tile_pool(name="psA", bufs=3, space="PSUM"))
        psT = actx.enter_context(tc.tile_pool(name="psT", bufs=2, space="PSUM"))

        araw = apool.tile([128, S - 1], f32, tag="araw", name="araw")
        nc.sync.dma_start(araw[:], P["alpha"][:])
        nc.scalar.activation(alphasig[:], araw[:], AF.Sigmoid)

        wq, wk, wv = [], [], []
        for k in range(NKT // 2):
            for (lst, name) in ((wq, "WqT"), (wk, "WkT"), (wv, "WvT")):
                w = w_pool.tile([128, DLOC], f32r, tag=f"w{name}{k}", name=f"w{name}{k}")
                nc.sync.dma_start(w[:], P[name][k * 128:(k + 1) * 128, :].bitcast(f32r))
                lst.append(w)

        for b in range(B):
            xt = []
            for k in range(8):
                t = xt_pool.tile([128, S], f32r, tag="xt", name="xt")
                nc.sync.dma_start(t[:], P["XT"][b, k * 128:(k + 1) * 128, :].bitcast(f32r))
                xt.append(t)

            # Q^T and K^T projections (out = [e(128), s])
            ktraw = apool.tile([128, S], f32, tag="ktraw", name="ktraw")
            for wt, is_q in ((wq, True), (wk, False)):
                for s4 in range(NSB):
                    pp = psA.tile([128, SB], f32, tag="projps", name="projps")
                    for k in range(8):
                        nc.tensor.matmul(pp[:], wt[k][:], xt[k][:, s4 * SB:(s4 + 1) * SB],
                                         start=(k == 0), stop=(k == 7))
                    sl = slice(s4 * SB, (s4 + 1) * SB)
                    if is_q:
                        nc.vector.tensor_add(QTf[b][:, sl], pp[:], peT[:, sl])
                    else:
                        nc.vector.tensor_copy(ktraw[:, sl], pp[:])

            # EMA smear on K + pos-emb -> K2 (f32r)
            nc.vector.tensor_add(K2[b][:, 0:1], ktraw[:, 0:1], peT[:, 0:1])
            for c4 in range(NSB):
                lo = max(1, c4 * SB)
                hi = (c4 + 1) * SB
                n = hi - lo
                t1 = apool.tile([128, SB], f32, tag="smr1", name="smr1")
                nc.vector.tensor_sub(t1[:, :n], ktraw[:, lo:hi], ktraw[:, lo - 1:hi - 1])
                t2 = apool.tile([128, SB], f32, tag="smr2", name="smr2")
                nc.vector.tensor_mul(t2[:, :n], t1[:, :n], alphasig[:, lo - 1:hi - 1])
                nc.vector.tensor_add(t1[:, :n], t2[:, :n], peT[:, lo:hi])
                nc.vector.tensor_add(K2[b][:, lo:hi], t1[:, :n], ktraw[:, lo - 1:hi - 1])

            # V^T projection then PE-transpose into V_aug tiles [s(128), h*(dk+1)]
            for s4 in range(NSB):
                pp = psA.tile([128, SB], f32, tag="projps", name="projps")
                for k in range(8):
                    nc.tensor.matmul(pp[:], wv[k][:], xt[k][:, s4 * SB:(s4 + 1) * SB],
                                     start=(k == 0), stop=(k == 7))
                vt = apool.tile([128, SB], f32, tag="vt", name="vt")
                nc.vector.tensor_copy(vt[:], pp[:])
                for j in range(SB // 128):
                    tp = psT.tile([128, 128], f32, tag="tp", name="tp")
                    nc.tensor.transpose(tp[:], vt[:, j * 128:(j + 1) * 128], ident[:])
                    va = vaug[b][s4 * 4 + j]
                    dst = va[:].rearrange("p (h w) -> p h w", h=HPC)
                    nc.vector.tensor_copy(dst[:, :, 0:DK],
                                          tp[:].rearrange("p (h w) -> p h w", h=HPC))
                    nc.vector.tensor_copy(dst[:, :, DK:DK + 1],
                                          ones_f[:].rearrange("p (h o) -> p h o", o=1))

    # ---------------- phase B: attention ----------------
    attnT = [None] * B
    at_pool = ctx.enter_context(tc.tile_pool(name="attnT", bufs=2))
    for b in range(B):
        attnT[b] = at_pool.tile([128, S], f32r, tag="attnT", name="attnT")

    with contextlib.ExitStack() as bctx:
        pt_pool = bctx.enter_context(tc.tile_pool(name="pt", bufs=3))
        rc_pool = bctx.enter_context(tc.tile_pool(name="rc", bufs=2))
        psS = bctx.enter_context(tc.tile_pool(name="psS", bufs=3, space="PSUM"))
        psO = bctx.enter_context(tc.tile_pool(name="psO", bufs=2, space="PSUM"))
        psR = bctx.enter_context(tc.tile_pool(name="psR", bufs=2, space="PSUM"))

        for b in range(B):
            for h in range(HPC):
                hsl = slice(DK * h, DK * (h + 1))
                vsl = slice((DK + 1) * h, (DK + 1) * (h + 1))
                for qb in range(NSB):
                    nk = 4 * (qb + 1)
                    ops = psO.tile([DK + 1, SB], f32, tag="ops", name="ops")
                    for kt in range(nk):
                        sps = psS.tile([128, SB], f32, tag="sps", name="sps")
                        nc.tensor.matmul(
                            sps[:],
                            K2[b][hsl, kt * 128:(kt + 1) * 128],
                            QTf[b][hsl, qb * SB:(qb + 1) * SB],
                            start=True, stop=True, skip_group_check=True)
                        pt = pt_pool.tile([128, SB], f32r, tag="pt", name="pt")
                        p = kt - 4 * qb
                        if p >= 0:
                            nc.scalar.activation(pt[:], sps[:], AF.Exp, scale=0.125)
                            nc.vector.tensor_mul(pt[:], pt[:].bitcast(f32), mask_sb[p][:])
                        else:
                            nc.scalar.activation(pt[:], sps[:], AF.Exp, scale=0.125)
                        nc.tensor.matmul(
                            ops[:], vaug[b][kt][:, vsl], pt[:],
                            start=(kt == 0), stop=(kt == nk - 1),
                            skip_group_check=True)
                    rec = rc_pool.tile([1, SB], f32r, tag="rec", name="rec")
                    nc.vector.reciprocal(rec[:], ops[DK:DK + 1, :])
                    rps = psR.tile([DK, SB], f32, tag="rps", name="rps")
                    nc.tensor.matmul(rps[:], ones64[:], rec[:], start=True, stop=True,
                                     skip_group_check=True)
                    rsb = rc_pool.tile([DK, SB], f32, tag="rsb", name="rsb")
                    nc.vector.tensor_copy(rsb[:], rps[:])
                    nc.vector.tensor_mul(
                        attnT[b][hsl, qb * SB:(qb + 1) * SB],
                        ops[0:DK, :], rsb[:])

    # ---------------- all-to-all ----------------
    dram = ctx.enter_context(tc.tile_pool(name="dram", bufs=1, space="DRAM"))
    a2a_in = dram.tile([NCORES, 128, SB], f32, tag="a2ain", name="a2ain")
    a2a_out = dram.tile([NCORES, 128, SB], f32, tag="a2aout", name="a2aout")
    for j in range(NCORES):
        nc.sync.dma_start(a2a_in[j], attnT[j // 4][:, (j % 4) * SB:(j % 4 + 1) * SB].bitcast(f32))
    nc.gpsimd.collective_compute(
        "AllToAll", mybir.AluOpType.bypass,
        replica_groups=[list(range(NCORES))],
        ins=[a2a_in[:]], outs=[a2a_out[:]])

    # ---------------- phase C: output proj + residual + LN ----------------
    with contextlib.ExitStack() as cctx:
        atk_pool = cctx.enter_context(tc.tile_pool(name="atk", bufs=8))
        wo_pool = cctx.enter_context(tc.tile_pool(name="wo", bufs=8))
        xs_pool = cctx.enter_context(tc.tile_pool(name="xsl", bufs=4))
        y_pool = cctx.enter_context(tc.tile_pool(name="y", bufs=2))
        st_pool = cctx.enter_context(tc.tile_pool(name="lnstat", bufs=4))
        psC = cctx.enter_context(tc.tile_pool(name="psC", bufs=3, space="PSUM"))

        at = []
        for i in range(NCORES):
            t = atk_pool.tile([128, SB], f32r, tag="atk", name="atk")
            nc.sync.dma_start(t[:], a2a_out[i].bitcast(f32r))
            at.append(t)
        wo = []
        for k in range(8):
            t = wo_pool.tile([128, D], f32r, tag="wo", name="wo")
            nc.sync.dma_start(t[:], P["WoT"][k * 128:(k + 1) * 128, :].bitcast(f32r))
            wo.append(t)
        xsl = []
        for st in range(SB // 128):
            t = xs_pool.tile([128, D], f32, tag="xsl", name="xsl")
            nc.sync.dma_start(t[:], P["Xsl"][st * 128:(st + 1) * 128, :])
            xsl.append(t)

        for st in range(SB // 128):
            ysb = y_pool.tile([128, D], f32, tag="ysb", name="ysb")
            for nb in range(D // SB):
                yps = psC.tile([128, SB], f32, tag="yps", name="yps")
                for k in range(8):
                    nc.tensor.matmul(yps[:], at[k][:, st * 128:(st + 1) * 128],
                                     wo[k][:, nb * SB:(nb + 1) * SB],
                                     start=(k == 0), stop=(k == 7))
                nc.vector.tensor_add(ysb[:, nb * SB:(nb + 1) * SB], yps[:],
                                     xsl[st][:, nb * SB:(nb + 1) * SB])
            # LayerNorm over the free (feature) dim
            mu = st_pool.tile([128, 1], f32, tag="mu", name="mu")
            nc.vector.reduce_sum(mu[:], ysb[:], axis=AX)
            nc.scalar.mul(mu[:], mu[:], -1.0 / D)
            xc = y_pool.tile([128, D], f32, tag="xc", name="xc")
            nc.scalar.add(xc[:], ysb[:], mu[:])
            sq = y_pool.tile([128, D], f32, tag="sq", name="sq")
            nc.scalar.activation(sq[:], xc[:], AF.Square)
            var = st_pool.tile([128, 1], f32, tag="var", name="var")
            nc.vector.reduce_sum(var[:], sq[:], axis=AX)
            nc.scalar.mul(var[:], var[:], 1.0 / D)
            std = st_pool.tile([128, 1], f32, tag="std", name="std")
            nc.scalar.activation(std[:], var[:], AF.Sqrt, bias=eps_t[:])
            inv = st_pool.tile([128, 1], f32, tag="inv", name="inv")
            nc.vector.reciprocal(inv[:], std[:])
            nc.vector.tensor_mul(xc[:], xc[:], inv[:].to_broadcast((128, D)))
            nc.vector.tensor_mul(xc[:], xc[:], lnsc[:])
            nc.vector.tensor_add(xc[:], xc[:], lnb[:])
            nc.sync.dma_start(P["Y"][st * 128:(st + 1) * 128, :], xc[:])


def build(repeat=1):
    import contextlib
    nc = bacc.Bacc(num_devices=NCORES)
    P = {
        "XT": nc.declare_dram_parameter("XT", [B, D, S], f32, isOutput=False),
        "WqT": nc.declare_dram_parameter("WqT", [D, DLOC], f32, isOutput=False),
        "WkT": nc.declare_dram_parameter("WkT", [D, DLOC], f32, isOutput=False),
        "WvT": nc.declare_dram_parameter("WvT", [D, DLOC], f32, isOutput=False),
        "WoT": nc.declare_dram_parameter("WoT", [D, D], f32, isOutput=False),
        "Xsl": nc.declare_dram_parameter("Xsl", [SB, D], f32, isOutput=False),
        "alpha": nc.declare_dram_parameter("alpha", [128, S - 1], f32, isOutput=False),
        "peT": nc.declare_dram_parameter("peT", [128, S], f32, isOutput=False),
        "mask": nc.declare_dram_parameter("mask", [4, KTILE, SB], f32, isOutput=False),
        "lnsc": nc.declare_dram_parameter("lnsc", [128, D], f32, isOutput=False),
        "lnb": nc.declare_dram_parameter("lnb", [128, D], f32, isOutput=False),
        "Y": nc.declare_dram_parameter("Y", [SB, D], f32, isOutput=True),
    }
    with tile.TileContext(nc) as tc:
        for _ in range(repeat):
            with contextlib.ExitStack() as ctx:
                _emit(nc, tc, P, ctx)
    nc.finalize()
    return nc


def _pe_table():
    pos = np.arange(S, dtype=np.float32)[:, None]
    i = np.arange(DK // 2, dtype=np.float32)[None, :]
    freq = np.exp(-(2.0 * i / DK) * np.log(10000.0)).astype(np.float32)
    ang = pos * freq
    pe = np.zeros((S, DK), dtype=np.float32)
    pe[:, 0::2] = np.sin(ang)
    pe[:, 1::2] = np.cos(ang)
    return pe


def make_in_maps(X, W_q, W_k, W_v, W_o, alpha, ln_scale, ln_bias):
    XT = np.ascontiguousarray(X.transpose(0, 2, 1))
    WoT = np.ascontiguousarray(W_o.T)
    pe = _pe_table()
    peT = np.ascontiguousarray(np.vstack([pe.T, pe.T]))
    qq = np.arange(SB, dtype=np.float32)[None, :]
    masks = np.stack([
        (qq >= (p * KTILE + np.arange(KTILE, dtype=np.float32)[:, None]))
        .astype(np.float32) for p in range(4)])
    in_maps = []
    for c in range(NCORES):
        rsl = slice(DLOC * c, DLOC * (c + 1))
        al = alpha[0, HPC * c:HPC * (c + 1), :, 0]              # [HPC, S-1]
        al_b = np.ascontiguousarray(np.repeat(al, DK, axis=0))   # [128, S-1]
        bc, sc = c // 4, c % 4
        in_maps.append({
            "XT": XT,
            "WqT": np.ascontiguousarray(W_q[rsl, :].T),
            "WkT": np.ascontiguousarray(W_k[rsl, :].T),
            "WvT": np.ascontiguousarray(W_v[rsl, :].T),
            "WoT": WoT,
            "Xsl": np.ascontiguousarray(X[bc, sc * SB:(sc + 1) * SB, :]),
            "alpha": al_b,
            "peT": peT,
            "mask": masks,
            "lnsc": np.ascontiguousarray(np.broadcast_to(ln_scale, (128, D))),
            "lnb": np.ascontiguousarray(np.broadcast_to(ln_bias, (128, D))),
        })
    return in_maps


_NC_CACHE = {}


def get_nc(repeat=1):
    if repeat not in _NC_CACHE:
        _NC_CACHE[repeat] = build(repeat)
    return _NC_CACHE[repeat]


def kernel(X, W_q, W_k, W_v, W_o, alpha, ln_scale, ln_bias):
    args = [np.ascontiguousarray(np.asarray(a, dtype=np.float32))
            for a in (X, W_q, W_k, W_v, W_o, alpha, ln_scale, ln_bias)]
    in_maps = make_in_maps(*args)
    nc = get_nc(1)
    res = run_bass_kernel_spmd(nc, in_maps, list(range(NCORES)))
    Y = np.empty((B, S, D), dtype=np.float32)
    for c in range(NCORES):
        bc, sc = c // 4, c % 4
        Y[bc, sc * SB:(sc + 1) * SB, :] = res.results[c]["Y"]
    return Y
